# revision 19
# baseline (speedup 1.0000x reference)
"""Multi-head causal attention (B=4, T=2048, D=1024, H=16, HS=64) on 8 TRN2
NeuronCores.

Sharding: batch (4-way) x head-group (2-way).  Core c handles batch c//2 and
heads 8*(c%2) .. 8*(c%2)+7.  Each core computes its 8 heads' attention and the
partial output projection Y_T = sum_h Wo_h^T @ O_T_h; the host sums the two
head-group partials per batch, transposes, and adds the output bias.

v2 structure (vs the v1 baseline):
  - V is produced directly in [t, e] layout (lhsT = x^T chunks, rhs = all 8
    heads' Wv columns), which removes all 128 PE transposes.
  - Scores S^T [k, q] contract over e=64, so the two heads of a pair are
    row-packed: head 0 on PE rows 0-63, head 1 on rows 64-127 (tile_position
    auto-derived from base partitions).  Adjacent emission makes the two
    matmuls run concurrently -> ~2x on the S stream.
  - S lands in PSUM as bf16: one 2KB bank holds a [128, 2, 512] slot (two
    k-chunks), so exp runs as [128, ~1024] activations and the whole
    attention pipeline fits in 8 banks: 4 x S (2 heads x 2 slots in flight)
    + 2 x O accumulators + 2 x general matmul banks.
  - The output projection runs as filler inside pair 3's attention: for each
    (dc, qc) all four pair contributions accumulate in one PSUM bank, then a
    single copy + DMA out.  Q/K projections for pair p+1 and the remaining
    V-projection chunks fill pairs 0-2.
  - ScalarE does (almost) only exp; psum evacuations and the softmax
    normalization (1/l broadcast via a DRAM bounce) run on VectorE in bf16.
"""

import numpy as np

B, T, D = 4, 2048, 1024
H, HS = 16, 64
NCORES = 8
NPAIR = 4   # head pairs per core
ND = 8      # 128-wide d chunks
NT = 16     # 128-wide t chunks
NQ = 4      # 512-wide q chunks
NK = 16     # 128-wide k chunks

_CACHE = {}


def _build_program(dbg=False):
    import concourse.bass as bass
    import concourse.tile as tile
    from concourse import bacc, mybir
    from contextlib import ExitStack

    f32 = mybir.dt.float32
    bf16 = mybir.dt.bfloat16
    f8 = mybir.dt.float8e4
    Exp = mybir.ActivationFunctionType.Exp
    DR = mybir.MatmulPerfMode.DoubleRow

    nc = bacc.Bacc("TRN2", target_bir_lowering=False, debug=False)

    x_d = nc.declare_dram_parameter("x", [128, NQ, ND, 512], bf16, isOutput=False)
    wq_d = nc.declare_dram_parameter("wq", [NPAIR, 128, ND, 128], bf16, isOutput=False)
    wk_d = nc.declare_dram_parameter("wk", [NPAIR, 128, ND, 128], bf16, isOutput=False)
    wv_d = nc.declare_dram_parameter("wv", [128, ND, 512], bf16, isOutput=False)
    wo_d = nc.declare_dram_parameter("wo", [128, NPAIR, ND, 128], bf16, isOutput=False)
    tri_d = nc.declare_dram_parameter("tri", [128, 128], bf16, isOutput=False)
    yt_d = nc.declare_dram_parameter("yt", [D, T], f32, isOutput=True)
    if dbg:
        qt_dbg = nc.declare_dram_parameter("qt_dbg", [128, NPAIR, T], bf16, isOutput=True)
        kt_dbg = nc.declare_dram_parameter("kt_dbg", [128, NPAIR, T], bf16, isOutput=True)
        va_dbg = nc.declare_dram_parameter("va_dbg", [128, NT, 8, 65], bf16, isOutput=True)
        ot_dbg = nc.declare_dram_parameter("ot_dbg", [128, NPAIR, T], bf16, isOutput=True)
        pt_dbg = nc.declare_dram_parameter("pt_dbg", [4, 128, 2, 512], bf16, isOutput=True)
        oc_dbg = nc.declare_dram_parameter("oc_dbg", [2, 64, 512], f32, isOutput=True)
        rl_dbg = nc.declare_dram_parameter("rl_dbg", [2, 1, 512], f32, isOutput=True)
        lb_dbg = nc.declare_dram_parameter("lb_dbg", [2, 64, 512], f32, isOutput=True)
        dbg_state = {"pt": [], "norm": []}

    with tile.TileContext(nc) as tc, ExitStack() as top:
        const = top.enter_context(tc.tile_pool(name="const", bufs=1))
        ones64 = const.tile([1, 64], f32, name="ones64")
        nc.vector.memset(ones64, 1.0)
        # tri2[:, h, :] = upper-triangular causal mask, replicated per head so
        # one DVE mul masks both heads of a pair
        tri2 = const.tile([128, 2, 128], bf16, name="tri2")
        nc.sync.dma_start(out=tri2[:, 0, :], in_=tri_d[:, :])
        nc.sync.dma_start(out=tri2[:, 1, :], in_=tri_d[:, :])
        # touch Exp early so the ~2.7us ACT table load overlaps phase A
        scr = const.tile([1, 8], bf16, name="scr")
        nc.scalar.activation(out=scr, in_=tri2[0:1, 0, 0:8], func=Exp, scale=1.0)

        big = top.enter_context(tc.tile_pool(name="big", bufs=1))
        # vaug[:, c, h, 0:64] = V[t=c*128..+128, e=h*64..+64]; col 64 = ones
        vaug = big.tile([128, NT, 8, 65], bf16, name="vaug")
        nc.vector.memset(vaug[:, :, :, 64:65], 1.0)

        xtp = top.enter_context(tc.tile_pool(name="xtp", bufs=1))
        xt = xtp.tile([128, NQ, ND, 512], bf16, name="xt")
        wvp = top.enter_context(tc.tile_pool(name="wvp", bufs=1))
        wv_sb = wvp.tile([128, ND, 512], bf16, name="wv_sb")
        qkp = top.enter_context(tc.tile_pool(name="qkp", bufs=1))
        qt = qkp.tile([128, NPAIR, T], bf16, name="qt")
        kt = qkp.tile([128, NPAIR, T], bf16, name="kt")
        otn_p = top.enter_context(tc.tile_pool(name="otn_p", bufs=1))
        otn = otn_p.tile([128, NPAIR, T], bf16, name="otn")
        pwo = top.enter_context(tc.tile_pool(name="pwo", bufs=1))
        wo_sb = pwo.tile([128, NPAIR, ND, 128], bf16, name="wo_sb")

        pw = top.enter_context(tc.tile_pool(name="pw", bufs=4))
        ptp = top.enter_context(tc.tile_pool(name="ptp", bufs=4))
        ocp = top.enter_context(tc.tile_pool(name="ocp", bufs=2))
        rcp = top.enter_context(tc.tile_pool(name="rcp", bufs=2))
        lbp = top.enter_context(tc.tile_pool(name="lbp", bufs=2))
        pyt = top.enter_context(tc.tile_pool(name="pyt", bufs=3))
        drp = top.enter_context(tc.tile_pool(name="drp", bufs=4, space="DRAM"))

        # PSUM budget: psS 2x2 banks + psO 2 + psM 2 = 8
        psS = top.enter_context(tc.tile_pool(name="psS", bufs=2, space="PSUM"))
        psO = top.enter_context(tc.tile_pool(name="psO", bufs=2, space="PSUM"))
        psM = top.enter_context(tc.tile_pool(name="psM", bufs=2, space="PSUM"))

        # ---- DMA in first-use order (startup is DMA-latency-bound) ---------
        wq_sbs = [None] * NPAIR
        wk_sbs = [None] * NPAIR

        def dma_w(wdram, p, kind):
            w_sb = pw.tile([128, ND, 128], bf16, tag="w", name=f"w_{kind}{p}")
            nc.sync.dma_start(out=w_sb, in_=wdram[p])
            return w_sb

        wq_sbs[0] = dma_w(wq_d, 0, "q")
        wk_sbs[0] = dma_w(wk_d, 0, "k")
        nc.sync.dma_start(out=xt[:, 0, :, :], in_=x_d[:, 0, :, :])
        nc.sync.dma_start(out=wv_sb, in_=wv_d[:, :, :])
        for t4 in range(1, NQ):
            nc.sync.dma_start(out=xt[:, t4, :, :], in_=x_d[:, t4, :, :])
        nc.sync.dma_start(out=wo_sb, in_=wo_d[:, :, :, :])

        # ---- building blocks ----------------------------------------------
        def qk_proj_mms(ps, w_sb, t4, dc_lo, dc_hi):
            for dc in range(dc_lo, dc_hi):
                nc.tensor.matmul(
                    ps, w_sb[:, dc, :], xt[:, t4, dc, :],
                    start=(dc == 0), stop=(dc == ND - 1),
                )

        def v_proj(tc_):
            """V[t, e] for one 128-token chunk, all 8 heads at once."""
            t4, sub = tc_ // 4, tc_ % 4
            ps = psM.tile([128, 512], f32, tag="mm", name="psv")
            for dc in range(ND):
                nc.tensor.matmul(
                    ps,
                    xt[:, t4, dc, sub * 128:(sub + 1) * 128],
                    wv_sb[:, dc, :],
                    start=(dc == 0), stop=(dc == ND - 1),
                )
            nc.vector.tensor_copy(out=vaug[:, tc_, :, 0:64], in_=ps)

        # ---- Phase A: Q/K pair 0 + V chunks 0-3 (t4=0 work first) ----------
        def qk0(w_sb, dest, t4):
            ps = psM.tile([128, 512], f32, tag="mm", name="psqk")
            qk_proj_mms(ps, w_sb, t4, 0, ND)
            nc.scalar.copy(out=dest[:, 0, t4 * 512:(t4 + 1) * 512], in_=ps)

        qk0(wq_sbs[0], qt, 0)
        qk0(wk_sbs[0], kt, 0)
        for tc_ in range(4):
            v_proj(tc_)
        for t4 in range(1, NQ):
            qk0(wq_sbs[0], qt, t4)
            qk0(wk_sbs[0], kt, t4)

        # Deferred softmax-normalize multiplies: the 1/l partition-broadcast
        # rides a DRAM bounce; emitting the dependent DVE mul immediately
        # would head-of-line-block the in-order VectorE queue.  Each group
        # queues its two muls here; the next group flushes them (the DMA has
        # completed by then).
        pending_norm = []

        def flush_norm():
            while pending_norm:
                pending_norm.pop(0)()

        # ---- attention group for one head pair, one q-chunk ----------------
        def attn_group(p, j, filler):
            ncc = 4 * (j + 1)
            jw = j * 512
            flush_norm()
            po = [psO.tile([65, 512], f32, tag="O", name=f"po{h}")
                  for h in range(2)]
            pts = {}

            def off_of(c):
                sub = c - 4 * j
                return sub * 128 if 0 <= sub < 4 else 0

            def emit_s(c):
                # ps[:, h, :] spans two PSUM banks: the row-packed head
                # matmuls write different banks, one exp call reads both
                off = off_of(c)
                ps = psS.tile([128, 2, 512], f32, tag="S", name="ps")
                pt = ptp.tile([128, 2, 512], bf16, tag="pt", name="pt")
                for h in range(2):
                    e0 = h * 64
                    nc.tensor.matmul(
                        ps[:, h, off:],
                        kt[e0:e0 + 64, p, c * 128:(c + 1) * 128],
                        qt[e0:e0 + 64, p, jw + off:jw + 512],
                        start=True, stop=True,
                    )
                nc.scalar.activation(out=pt[:, :, off:], in_=ps[:, :, off:],
                                     func=Exp, scale=0.125)
                sub = c - 4 * j
                if 0 <= sub < 4:
                    nc.vector.tensor_mul(
                        pt[:, :, sub * 128:(sub + 1) * 128],
                        pt[:, :, sub * 128:(sub + 1) * 128],
                        tri2,
                    )
                if dbg and p == 0 and j == 0:
                    nc.sync.dma_start(out=pt_dbg[c], in_=pt)
                pts[c] = pt

            def emit_v(c):
                pt = pts.pop(c)
                off = off_of(c)
                for h in range(2):
                    nc.tensor.matmul(
                        po[h][:, off:],
                        vaug[:, c, 2 * p + h, :],
                        pt[:, h, off:],
                        start=(c == 0), stop=(c == ncc - 1),
                    )

            emit_s(0)
            if ncc > 1:
                emit_s(1)
            for c in range(ncc):
                if c + 2 < ncc:
                    emit_s(c + 2)
                if c % 2 == 0:
                    filler()
                emit_v(c)

            # normalize: otn[e, q] = O_T[e, q] / l[q] on DVE;
            # the final mul is deferred until the broadcast DMA has landed
            for h in range(2):
                oc = ocp.tile([64, 512], f32, tag="oc", name="oc")
                nc.vector.tensor_copy(out=oc, in_=po[h][0:64, :])
                rlc = rcp.tile([1, 512], f32, tag="rlc", name="rlc")
                nc.vector.tensor_copy(out=rlc, in_=po[h][64:65, :])
                rl = rcp.tile([1, 512], f32, tag="rl", name="rl")
                nc.vector.reciprocal_approx_fast(rl, rlc)
                lb = psM.tile([64, 512], f32, tag="mm", name="lb")
                nc.tensor.matmul(lb, ones64, rl, start=True, stop=True)
                e0 = h * 64

                def norm_mul(oc=oc, lb=lb, e0=e0, p=p, jw=jw):
                    nc.vector.tensor_mul(
                        otn[e0:e0 + 64, p, jw:jw + 512], oc, lb
                    )
                pending_norm.append(norm_mul)
                if dbg and p == 0 and j == 0:
                    nc.sync.dma_start(out=oc_dbg[h], in_=oc)
                    nc.sync.dma_start(out=rl_dbg[h], in_=rl)
                    nc.sync.dma_start(out=lb_dbg[h], in_=lb)

        # ---- filler units ---------------------------------------------------
        def mk_qk_unit(w_sb, dest, p, t4, dc_lo, dc_hi, state):
            def emit():
                if dc_lo == 0:
                    state["ps"] = psM.tile([128, 512], f32, tag="mm", name="psf")
                qk_proj_mms(state["ps"], w_sb, t4, dc_lo, dc_hi)
                if dc_hi == ND:
                    nc.vector.tensor_copy(
                        out=dest[:, p, t4 * 512:(t4 + 1) * 512],
                        in_=state["ps"])
            return emit

        def mk_out_unit(dc, qc):
            def emit():
                py = psM.tile([128, 512], f32, tag="mm", name="pyo")
                for pp in range(NPAIR):
                    nc.tensor.matmul(
                        py,
                        wo_sb[:, pp, dc, :],
                        otn[:, pp, qc * 512:(qc + 1) * 512],
                        start=(pp == 0), stop=(pp == NPAIR - 1),
                    )
                yt_sb = pyt.tile([128, 512], f32, tag="yt", name="yt_f")
                nc.vector.tensor_copy(out=yt_sb, in_=py)
                nc.sync.dma_start(
                    out=yt_d[dc * 128:(dc + 1) * 128,
                             qc * 512:(qc + 1) * 512],
                    in_=yt_sb,
                )
            return emit

        # ---- Phase B: four pair phases -------------------------------------
        for p in range(NPAIR):
            fill = []
            if p == 0:
                # remaining V chunks first (rate-2 filler covers readiness:
                # j=1 needs tc<8 after j=0's fillers, etc.), then QK pair 1
                for tc_ in range(4, NT):
                    fill.append(lambda tc_=tc_: v_proj(tc_))
            if p < NPAIR - 1:
                wq_sbs[p + 1] = dma_w(wq_d, p + 1, "q")
                wk_sbs[p + 1] = dma_w(wk_d, p + 1, "k")
                for w_sb, dest in ((wq_sbs[p + 1], qt), (wk_sbs[p + 1], kt)):
                    for t4 in range(NQ):
                        state = {}
                        for dc_lo in (0, 4):
                            fill.append(mk_qk_unit(w_sb, dest, p + 1, t4,
                                                   dc_lo, dc_lo + 4, state))

            def filler(fill=fill):
                n = 2 if fill and len(fill) > 8 else 1
                for _ in range(n):
                    if fill:
                        fill.pop(0)()

            if p < NPAIR - 1:
                for j in range(NQ):
                    attn_group(p, j, filler)
            else:
                # pair 3: j descending, so the tail waits only on the
                # shortest group's normalize; out-units follow completions
                for j in (3, 2, 1, 0):
                    if j < 3:
                        qc = j + 1
                        for dc in range(ND):
                            fill.append(mk_out_unit(dc, qc))
                    attn_group(p, j, filler)
            while fill:
                fill.pop(0)()

        # tail: q-chunk 0 of the output projection
        flush_norm()
        for dc in range(ND):
            mk_out_unit(dc, 0)()

        if dbg:
            nc.sync.dma_start(out=qt_dbg[:, :, :], in_=qt)
            nc.sync.dma_start(out=kt_dbg[:, :, :], in_=kt)
            nc.sync.dma_start(out=va_dbg[:, :, :, :], in_=vaug)
            nc.sync.dma_start(out=ot_dbg[:, :, :], in_=otn)

    nc.compile()
    return nc


def _pack_inputs(x, Wq, Wk, Wv, Wo):
    """Per-core input maps. Core c: batch c//2, head group c%2."""
    import ml_dtypes

    tri = np.triu(np.ones((128, 128), np.float32)).astype(ml_dtypes.bfloat16)

    def pack_w(W, g):
        # [NPAIR, 128(d_local), ND, 128(e2)]
        out = np.empty((NPAIR, 128, ND, 128), np.float32)
        for p in range(NPAIR):
            h1 = 8 * g + 2 * p
            r = W[[h1, h1 + 1]].transpose(1, 0, 2).reshape(D, 128)  # [d, e2]
            out[p] = r.reshape(ND, 128, 128).transpose(1, 0, 2)
        return np.ascontiguousarray(out).astype(ml_dtypes.bfloat16)

    def pack_wv(W, g):
        # [128(d within chunk), ND, 512(e = h*64+hs over 8 heads)]
        r = W[8 * g:8 * g + 8].transpose(1, 0, 2).reshape(D, 512)  # [d, e]
        out = r.reshape(ND, 128, 512).transpose(1, 0, 2)
        return np.ascontiguousarray(out).astype(ml_dtypes.bfloat16)

    def pack_wo(Wo, g):
        # [128(e2), NPAIR, ND, 128(d)]
        out = np.empty((128, NPAIR, ND, 128), np.float32)
        for p in range(NPAIR):
            r0 = (8 * g + 2 * p) * 64
            out[:, p] = Wo[r0:r0 + 128].reshape(128, ND, 128)
        return np.ascontiguousarray(out).astype(ml_dtypes.bfloat16)

    packs = {}
    for g in range(2):
        packs[g] = dict(
            wq=pack_w(Wq, g), wk=pack_w(Wk, g), wv=pack_wv(Wv, g),
            wo=pack_wo(Wo, g),
        )
    in_maps = []
    for c in range(NCORES):
        b, g = c // 2, c % 2
        m = dict(packs[g])
        xt = x[b].reshape(NQ, 512, ND, 128).transpose(3, 0, 2, 1)
        m["x"] = np.ascontiguousarray(xt).astype(ml_dtypes.bfloat16)
        m["tri"] = tri
        in_maps.append(m)
    return in_maps


def kernel(x, Wq, Wk, Wv, Wo, bo):
    from concourse.bass_utils import run_bass_kernel_spmd

    x = np.asarray(x, np.float32)
    Wq, Wk, Wv = (np.asarray(a, np.float32) for a in (Wq, Wk, Wv))
    Wo = np.asarray(Wo, np.float32)
    bo = np.asarray(bo, np.float32)

    if "nc" not in _CACHE:
        _CACHE["nc"] = _build_program()
    nc = _CACHE["nc"]

    in_maps = _pack_inputs(x, Wq, Wk, Wv, Wo)
    res = run_bass_kernel_spmd(nc, in_maps, list(range(NCORES)))
    _CACHE["last_result"] = res

    out = np.empty((B, T, D), np.float32)
    for b in range(B):
        yt = res.results[2 * b]["yt"] + res.results[2 * b + 1]["yt"]
        out[b] = yt.T + bo
    return out


# revision 20
# speedup vs baseline: 1.1958x; 1.1958x over previous
"""Multi-head causal attention (B=4, T=2048, D=1024, H=16, HS=64) on 8 TRN2
NeuronCores.

Sharding: batch (4-way) x head-group (2-way).  Core c handles batch c//2 and
heads 8*(c%2) .. 8*(c%2)+7.  Each core computes its 8 heads' attention and the
partial output projection Y_T = sum_h Wo_h^T @ O_T_h; the host sums the two
head-group partials per batch, transposes, and adds the output bias.

v2 structure (vs the v1 baseline):
  - V is produced directly in [t, e] layout (lhsT = x^T chunks, rhs = all 8
    heads' Wv columns), which removes all 128 PE transposes.
  - Scores S^T [k, q] contract over e=64, so the two heads of a pair are
    row-packed: head 0 on PE rows 0-63, head 1 on rows 64-127 (tile_position
    auto-derived from base partitions).  Adjacent emission makes the two
    matmuls run concurrently -> ~2x on the S stream.
  - S lands in PSUM as bf16: one 2KB bank holds a [128, 2, 512] slot (two
    k-chunks), so exp runs as [128, ~1024] activations and the whole
    attention pipeline fits in 8 banks: 4 x S (2 heads x 2 slots in flight)
    + 2 x O accumulators + 2 x general matmul banks.
  - The output projection runs as filler inside pair 3's attention: for each
    (dc, qc) all four pair contributions accumulate in one PSUM bank, then a
    single copy + DMA out.  Q/K projections for pair p+1 and the remaining
    V-projection chunks fill pairs 0-2.
  - ScalarE does (almost) only exp; psum evacuations and the softmax
    normalization (1/l broadcast via a DRAM bounce) run on VectorE in bf16.
"""

import numpy as np

B, T, D = 4, 2048, 1024
H, HS = 16, 64
NCORES = 8
NPAIR = 4   # head pairs per core
ND = 8      # 128-wide d chunks
NT = 16     # 128-wide t chunks
NQ = 4      # 512-wide q chunks
NK = 16     # 128-wide k chunks

_CACHE = {}


def _build_program(dbg=False):
    import concourse.bass as bass
    import concourse.tile as tile
    from concourse import bacc, mybir
    from contextlib import ExitStack

    f32 = mybir.dt.float32
    bf16 = mybir.dt.bfloat16
    f8 = mybir.dt.float8e4
    Exp = mybir.ActivationFunctionType.Exp
    DR = mybir.MatmulPerfMode.DoubleRow

    nc = bacc.Bacc("TRN2", target_bir_lowering=False, debug=False)

    x_d = nc.declare_dram_parameter("x", [128, NQ, ND, 512], bf16, isOutput=False)
    wq_d = nc.declare_dram_parameter("wq", [NPAIR, 128, ND, 128], bf16, isOutput=False)
    wk_d = nc.declare_dram_parameter("wk", [NPAIR, 128, ND, 128], bf16, isOutput=False)
    wv_d = nc.declare_dram_parameter("wv", [128, ND, 512], bf16, isOutput=False)
    wo_d = nc.declare_dram_parameter("wo", [128, NPAIR, ND, 128], bf16, isOutput=False)
    tri_d = nc.declare_dram_parameter("tri", [128, 128], bf16, isOutput=False)
    yt_d = nc.declare_dram_parameter("yt", [D, T], f32, isOutput=True)
    if dbg:
        qt_dbg = nc.declare_dram_parameter("qt_dbg", [128, NPAIR, T], bf16, isOutput=True)
        kt_dbg = nc.declare_dram_parameter("kt_dbg", [128, NPAIR, T], bf16, isOutput=True)
        va_dbg = nc.declare_dram_parameter("va_dbg", [128, NT, 8, 65], bf16, isOutput=True)
        ot_dbg = nc.declare_dram_parameter("ot_dbg", [128, NPAIR, T], bf16, isOutput=True)
        pt_dbg = nc.declare_dram_parameter("pt_dbg", [4, 128, 2, 512], bf16, isOutput=True)
        oc_dbg = nc.declare_dram_parameter("oc_dbg", [2, 64, 512], f32, isOutput=True)
        rl_dbg = nc.declare_dram_parameter("rl_dbg", [2, 1, 512], f32, isOutput=True)
        lb_dbg = nc.declare_dram_parameter("lb_dbg", [2, 64, 512], f32, isOutput=True)
        dbg_state = {"pt": [], "norm": []}

    with tile.TileContext(nc) as tc, ExitStack() as top:
        const = top.enter_context(tc.tile_pool(name="const", bufs=1))
        ones64 = const.tile([1, 64], f32, name="ones64")
        nc.vector.memset(ones64, 1.0)
        # tri2[:, h, :] = upper-triangular causal mask, replicated per head so
        # one DVE mul masks both heads of a pair
        tri2 = const.tile([128, 2, 128], bf16, name="tri2")
        nc.sync.dma_start(out=tri2[:, 0, :], in_=tri_d[:, :])
        nc.sync.dma_start(out=tri2[:, 1, :], in_=tri_d[:, :])
        # touch Exp early so the ~2.7us ACT table load overlaps phase A
        scr = const.tile([1, 8], bf16, name="scr")
        nc.scalar.activation(out=scr, in_=tri2[0:1, 0, 0:8], func=Exp, scale=1.0)

        big = top.enter_context(tc.tile_pool(name="big", bufs=1))
        # vaug[:, c, h, 0:64] = V[t=c*128..+128, e=h*64..+64]; col 64 = ones
        vaug = big.tile([128, NT, 8, 65], bf16, name="vaug")
        nc.vector.memset(vaug[:, :, :, 64:65], 1.0)

        xtp = top.enter_context(tc.tile_pool(name="xtp", bufs=1))
        xt = xtp.tile([128, NQ, ND, 512], bf16, name="xt")
        wvp = top.enter_context(tc.tile_pool(name="wvp", bufs=1))
        wv_sb = wvp.tile([128, ND, 512], bf16, name="wv_sb")
        qkp = top.enter_context(tc.tile_pool(name="qkp", bufs=1))
        qt = qkp.tile([128, NPAIR, T], bf16, name="qt")
        kt = qkp.tile([128, NPAIR, T], bf16, name="kt")
        otn_p = top.enter_context(tc.tile_pool(name="otn_p", bufs=1))
        otn = otn_p.tile([128, NPAIR, T], bf16, name="otn")
        pwo = top.enter_context(tc.tile_pool(name="pwo", bufs=1))
        wo_sb = pwo.tile([128, NPAIR, ND, 128], bf16, name="wo_sb")

        pw = top.enter_context(tc.tile_pool(name="pw", bufs=4))
        ptp = top.enter_context(tc.tile_pool(name="ptp", bufs=4))
        ocp = top.enter_context(tc.tile_pool(name="ocp", bufs=2))
        rcp = top.enter_context(tc.tile_pool(name="rcp", bufs=2))
        lbp = top.enter_context(tc.tile_pool(name="lbp", bufs=2))
        pyt = top.enter_context(tc.tile_pool(name="pyt", bufs=3))
        drp = top.enter_context(tc.tile_pool(name="drp", bufs=4, space="DRAM"))

        # PSUM budget: psS 2x2 banks + psO 2 + psM 2 = 8
        psS = top.enter_context(tc.tile_pool(name="psS", bufs=2, space="PSUM"))
        psO = top.enter_context(tc.tile_pool(name="psO", bufs=2, space="PSUM"))
        psM = top.enter_context(tc.tile_pool(name="psM", bufs=2, space="PSUM"))

        # ---- DMA in first-use order (startup is DMA-latency-bound) ---------
        wq_sbs = [None] * NPAIR
        wk_sbs = [None] * NPAIR

        def dma_w(wdram, p, kind):
            w_sb = pw.tile([128, ND, 128], bf16, tag="w", name=f"w_{kind}{p}")
            nc.sync.dma_start(out=w_sb, in_=wdram[p])
            return w_sb

        wq_sbs[0] = dma_w(wq_d, 0, "q")
        wk_sbs[0] = dma_w(wk_d, 0, "k")
        nc.sync.dma_start(out=xt[:, 0, :, :], in_=x_d[:, 0, :, :])
        nc.sync.dma_start(out=wv_sb, in_=wv_d[:, :, :])
        for t4 in range(1, NQ):
            nc.sync.dma_start(out=xt[:, t4, :, :], in_=x_d[:, t4, :, :])
        nc.sync.dma_start(out=wo_sb, in_=wo_d[:, :, :, :])

        # ---- building blocks ----------------------------------------------
        def qk_proj_mms(ps, w_sb, t4, dc_lo, dc_hi):
            for dc in range(dc_lo, dc_hi):
                nc.tensor.matmul(
                    ps, w_sb[:, dc, :], xt[:, t4, dc, :],
                    start=(dc == 0), stop=(dc == ND - 1),
                )

        def v_proj(tc_):
            """V[t, e] for one 128-token chunk, all 8 heads at once."""
            t4, sub = tc_ // 4, tc_ % 4
            ps = psM.tile([128, 512], f32, tag="mm", name="psv")
            for dc in range(ND):
                nc.tensor.matmul(
                    ps,
                    xt[:, t4, dc, sub * 128:(sub + 1) * 128],
                    wv_sb[:, dc, :],
                    start=(dc == 0), stop=(dc == ND - 1),
                )
            nc.vector.tensor_copy(out=vaug[:, tc_, :, 0:64], in_=ps)

        # ---- Phase A: Q/K pair 0 + V chunks 0-3 (t4=0 work first) ----------
        def qk0(w_sb, dest, t4):
            ps = psM.tile([128, 512], f32, tag="mm", name="psqk")
            qk_proj_mms(ps, w_sb, t4, 0, ND)
            nc.scalar.copy(out=dest[:, 0, t4 * 512:(t4 + 1) * 512], in_=ps)

        qk0(wq_sbs[0], qt, 0)
        qk0(wk_sbs[0], kt, 0)
        for tc_ in range(4):
            v_proj(tc_)
        for t4 in range(1, NQ):
            qk0(wq_sbs[0], qt, t4)
            qk0(wk_sbs[0], kt, t4)

        # Deferred softmax-normalize multiplies: the 1/l partition-broadcast
        # rides a DRAM bounce; emitting the dependent DVE mul immediately
        # would head-of-line-block the in-order VectorE queue.  Each group
        # queues its two muls here; the next group flushes them (the DMA has
        # completed by then).
        pending_norm = []

        def flush_norm():
            while pending_norm:
                pending_norm.pop(0)()

        # ---- attention group for one head pair, one q-chunk ----------------
        def attn_group(p, j, filler):
            ncc = 4 * (j + 1)
            jw = j * 512
            flush_norm()
            po = [psO.tile([65, 512], f32, tag="O", name=f"po{h}")
                  for h in range(2)]
            pts = {}

            def off_of(c):
                sub = c - 4 * j
                return sub * 128 if 0 <= sub < 4 else 0

            def emit_s(c):
                # ps[:, h, :] spans two PSUM banks: the row-packed head
                # matmuls write different banks, one exp call reads both
                off = off_of(c)
                ps = psS.tile([128, 2, 512], f32, tag="S", name="ps")
                pt = ptp.tile([128, 2, 512], bf16, tag="pt", name="pt")
                for h in range(2):
                    e0 = h * 64
                    nc.tensor.matmul(
                        ps[:, h, off:],
                        kt[e0:e0 + 64, p, c * 128:(c + 1) * 128],
                        qt[e0:e0 + 64, p, jw + off:jw + 512],
                        start=True, stop=True,
                    )
                nc.scalar.activation(out=pt[:, :, off:], in_=ps[:, :, off:],
                                     func=Exp, scale=0.125)
                sub = c - 4 * j
                if 0 <= sub < 4:
                    nc.vector.tensor_mul(
                        pt[:, :, sub * 128:(sub + 1) * 128],
                        pt[:, :, sub * 128:(sub + 1) * 128],
                        tri2,
                    )
                if dbg and p == 0 and j == 0:
                    nc.sync.dma_start(out=pt_dbg[c], in_=pt)
                pts[c] = pt

            def emit_v(c):
                pt = pts.pop(c)
                off = off_of(c)
                for h in range(2):
                    nc.tensor.matmul(
                        po[h][:, off:],
                        vaug[:, c, 2 * p + h, :],
                        pt[:, h, off:],
                        start=(c == 0), stop=(c == ncc - 1),
                    )

            emit_s(0)
            if ncc > 1:
                emit_s(1)
            for c in range(ncc):
                if c + 2 < ncc:
                    emit_s(c + 2)
                if c % 2 == 0:
                    filler()
                emit_v(c)

            # normalize: otn[e, q] = O_T[e, q] / l[q] on DVE;
            # the final mul is deferred until the broadcast DMA has landed
            for h in range(2):
                oc = ocp.tile([64, 512], f32, tag="oc", name="oc")
                nc.vector.tensor_copy(out=oc, in_=po[h][0:64, :])
                rlc = rcp.tile([1, 512], f32, tag="rlc", name="rlc")
                nc.vector.tensor_copy(out=rlc, in_=po[h][64:65, :])
                rl = rcp.tile([1, 512], f32, tag="rl", name="rl")
                nc.vector.reciprocal_approx_fast(rl, rlc)
                lb = lbp.tile([64, 512], f32, tag="lb", name="lb")
                nc.gpsimd.partition_broadcast(lb, rl)
                e0 = h * 64

                def norm_mul(oc=oc, lb=lb, e0=e0, p=p, jw=jw):
                    nc.vector.tensor_mul(
                        otn[e0:e0 + 64, p, jw:jw + 512], oc, lb
                    )
                pending_norm.append(norm_mul)
                if dbg and p == 0 and j == 0:
                    nc.sync.dma_start(out=oc_dbg[h], in_=oc)
                    nc.sync.dma_start(out=rl_dbg[h], in_=rl)
                    nc.sync.dma_start(out=lb_dbg[h], in_=lb)

        # ---- filler units ---------------------------------------------------
        def mk_qk_unit(w_sb, dest, p, t4, dc_lo, dc_hi, state):
            def emit():
                if dc_lo == 0:
                    state["ps"] = psM.tile([128, 512], f32, tag="mm", name="psf")
                qk_proj_mms(state["ps"], w_sb, t4, dc_lo, dc_hi)
                if dc_hi == ND:
                    nc.vector.tensor_copy(
                        out=dest[:, p, t4 * 512:(t4 + 1) * 512],
                        in_=state["ps"])
            return emit

        def mk_out_unit(dc, qc):
            def emit():
                py = psM.tile([128, 512], f32, tag="mm", name="pyo")
                for pp in range(NPAIR):
                    nc.tensor.matmul(
                        py,
                        wo_sb[:, pp, dc, :],
                        otn[:, pp, qc * 512:(qc + 1) * 512],
                        start=(pp == 0), stop=(pp == NPAIR - 1),
                    )
                yt_sb = pyt.tile([128, 512], f32, tag="yt", name="yt_f")
                nc.vector.tensor_copy(out=yt_sb, in_=py)
                nc.sync.dma_start(
                    out=yt_d[dc * 128:(dc + 1) * 128,
                             qc * 512:(qc + 1) * 512],
                    in_=yt_sb,
                )
            return emit

        # ---- Phase B: four pair phases -------------------------------------
        for p in range(NPAIR):
            fill = []
            if p == 0:
                # remaining V chunks first (rate-2 filler covers readiness:
                # j=1 needs tc<8 after j=0's fillers, etc.), then QK pair 1
                for tc_ in range(4, NT):
                    fill.append(lambda tc_=tc_: v_proj(tc_))
            if p < NPAIR - 1:
                wq_sbs[p + 1] = dma_w(wq_d, p + 1, "q")
                wk_sbs[p + 1] = dma_w(wk_d, p + 1, "k")
                for w_sb, dest in ((wq_sbs[p + 1], qt), (wk_sbs[p + 1], kt)):
                    for t4 in range(NQ):
                        state = {}
                        for dc_lo in (0, 4):
                            fill.append(mk_qk_unit(w_sb, dest, p + 1, t4,
                                                   dc_lo, dc_lo + 4, state))

            def filler(fill=fill):
                n = 2 if fill and len(fill) > 8 else 1
                for _ in range(n):
                    if fill:
                        fill.pop(0)()

            if p < NPAIR - 1:
                for j in range(NQ):
                    attn_group(p, j, filler)
            else:
                # pair 3: j descending, so the tail waits only on the
                # shortest group's normalize; out-units follow completions
                for j in (3, 2, 1, 0):
                    if j < 3:
                        qc = j + 1
                        for dc in range(ND):
                            fill.append(mk_out_unit(dc, qc))
                    attn_group(p, j, filler)
            while fill:
                fill.pop(0)()

        # tail: q-chunk 0 of the output projection
        flush_norm()
        for dc in range(ND):
            mk_out_unit(dc, 0)()

        if dbg:
            nc.sync.dma_start(out=qt_dbg[:, :, :], in_=qt)
            nc.sync.dma_start(out=kt_dbg[:, :, :], in_=kt)
            nc.sync.dma_start(out=va_dbg[:, :, :, :], in_=vaug)
            nc.sync.dma_start(out=ot_dbg[:, :, :], in_=otn)

    nc.compile()
    return nc


def _pack_inputs(x, Wq, Wk, Wv, Wo):
    """Per-core input maps. Core c: batch c//2, head group c%2."""
    import ml_dtypes

    tri = np.triu(np.ones((128, 128), np.float32)).astype(ml_dtypes.bfloat16)

    def pack_w(W, g):
        # [NPAIR, 128(d_local), ND, 128(e2)]
        out = np.empty((NPAIR, 128, ND, 128), np.float32)
        for p in range(NPAIR):
            h1 = 8 * g + 2 * p
            r = W[[h1, h1 + 1]].transpose(1, 0, 2).reshape(D, 128)  # [d, e2]
            out[p] = r.reshape(ND, 128, 128).transpose(1, 0, 2)
        return np.ascontiguousarray(out).astype(ml_dtypes.bfloat16)

    def pack_wv(W, g):
        # [128(d within chunk), ND, 512(e = h*64+hs over 8 heads)]
        r = W[8 * g:8 * g + 8].transpose(1, 0, 2).reshape(D, 512)  # [d, e]
        out = r.reshape(ND, 128, 512).transpose(1, 0, 2)
        return np.ascontiguousarray(out).astype(ml_dtypes.bfloat16)

    def pack_wo(Wo, g):
        # [128(e2), NPAIR, ND, 128(d)]
        out = np.empty((128, NPAIR, ND, 128), np.float32)
        for p in range(NPAIR):
            r0 = (8 * g + 2 * p) * 64
            out[:, p] = Wo[r0:r0 + 128].reshape(128, ND, 128)
        return np.ascontiguousarray(out).astype(ml_dtypes.bfloat16)

    packs = {}
    for g in range(2):
        packs[g] = dict(
            wq=pack_w(Wq, g), wk=pack_w(Wk, g), wv=pack_wv(Wv, g),
            wo=pack_wo(Wo, g),
        )
    in_maps = []
    for c in range(NCORES):
        b, g = c // 2, c % 2
        m = dict(packs[g])
        xt = x[b].reshape(NQ, 512, ND, 128).transpose(3, 0, 2, 1)
        m["x"] = np.ascontiguousarray(xt).astype(ml_dtypes.bfloat16)
        m["tri"] = tri
        in_maps.append(m)
    return in_maps


def kernel(x, Wq, Wk, Wv, Wo, bo):
    from concourse.bass_utils import run_bass_kernel_spmd

    x = np.asarray(x, np.float32)
    Wq, Wk, Wv = (np.asarray(a, np.float32) for a in (Wq, Wk, Wv))
    Wo = np.asarray(Wo, np.float32)
    bo = np.asarray(bo, np.float32)

    if "nc" not in _CACHE:
        _CACHE["nc"] = _build_program()
    nc = _CACHE["nc"]

    in_maps = _pack_inputs(x, Wq, Wk, Wv, Wo)
    res = run_bass_kernel_spmd(nc, in_maps, list(range(NCORES)))
    _CACHE["last_result"] = res

    out = np.empty((B, T, D), np.float32)
    for b in range(B):
        yt = res.results[2 * b]["yt"] + res.results[2 * b + 1]["yt"]
        out[b] = yt.T + bo
    return out


# revision 21
# speedup vs baseline: 1.2335x; 1.0315x over previous
"""Multi-head causal attention (B=4, T=2048, D=1024, H=16, HS=64) on 8 TRN2
NeuronCores.

Sharding: batch (4-way) x head-group (2-way).  Core c handles batch c//2 and
heads 8*(c%2) .. 8*(c%2)+7.  Each core computes its 8 heads' attention and the
partial output projection Y_T = sum_h Wo_h^T @ O_T_h; the host sums the two
head-group partials per batch, transposes, and adds the output bias.

v2 structure (vs the v1 baseline):
  - V is produced directly in [t, e] layout (lhsT = x^T chunks, rhs = all 8
    heads' Wv columns), which removes all 128 PE transposes.
  - Scores S^T [k, q] contract over e=64, so the two heads of a pair are
    row-packed: head 0 on PE rows 0-63, head 1 on rows 64-127 (tile_position
    auto-derived from base partitions).  Adjacent emission makes the two
    matmuls run concurrently -> ~2x on the S stream.
  - S lands in PSUM as bf16: one 2KB bank holds a [128, 2, 512] slot (two
    k-chunks), so exp runs as [128, ~1024] activations and the whole
    attention pipeline fits in 8 banks: 4 x S (2 heads x 2 slots in flight)
    + 2 x O accumulators + 2 x general matmul banks.
  - The output projection runs as filler inside pair 3's attention: for each
    (dc, qc) all four pair contributions accumulate in one PSUM bank, then a
    single copy + DMA out.  Q/K projections for pair p+1 and the remaining
    V-projection chunks fill pairs 0-2.
  - ScalarE does (almost) only exp; psum evacuations and the softmax
    normalization (1/l broadcast via a DRAM bounce) run on VectorE in bf16.
"""

import numpy as np

B, T, D = 4, 2048, 1024
H, HS = 16, 64
NCORES = 8
NPAIR = 4   # head pairs per core
ND = 8      # 128-wide d chunks
NT = 16     # 128-wide t chunks
NQ = 4      # 512-wide q chunks
NK = 16     # 128-wide k chunks

_CACHE = {}


def _build_program(dbg=False):
    import concourse.bass as bass
    import concourse.tile as tile
    from concourse import bacc, mybir
    from contextlib import ExitStack

    f32 = mybir.dt.float32
    bf16 = mybir.dt.bfloat16
    f8 = mybir.dt.float8e4
    Exp = mybir.ActivationFunctionType.Exp
    DR = mybir.MatmulPerfMode.DoubleRow

    nc = bacc.Bacc("TRN2", target_bir_lowering=False, debug=False)

    x_d = nc.declare_dram_parameter("x", [128, NQ, ND, 512], bf16, isOutput=False)
    wq_d = nc.declare_dram_parameter("wq", [NPAIR, 128, ND, 128], bf16, isOutput=False)
    wk_d = nc.declare_dram_parameter("wk", [NPAIR, 128, ND, 128], bf16, isOutput=False)
    wv_d = nc.declare_dram_parameter("wv", [128, ND, 512], bf16, isOutput=False)
    wo_d = nc.declare_dram_parameter("wo", [128, NPAIR, ND, 128], bf16, isOutput=False)
    tri_d = nc.declare_dram_parameter("tri", [128, 128], bf16, isOutput=False)
    yt_d = nc.declare_dram_parameter("yt", [D, T], bf16, isOutput=True)
    if dbg:
        qt_dbg = nc.declare_dram_parameter("qt_dbg", [128, NPAIR, T], bf16, isOutput=True)
        kt_dbg = nc.declare_dram_parameter("kt_dbg", [128, NPAIR, T], bf16, isOutput=True)
        va_dbg = nc.declare_dram_parameter("va_dbg", [128, NT, 8, 65], bf16, isOutput=True)
        ot_dbg = nc.declare_dram_parameter("ot_dbg", [128, NPAIR, T], bf16, isOutput=True)
        pt_dbg = nc.declare_dram_parameter("pt_dbg", [4, 128, 2, 512], bf16, isOutput=True)
        oc_dbg = nc.declare_dram_parameter("oc_dbg", [2, 64, 512], f32, isOutput=True)
        rl_dbg = nc.declare_dram_parameter("rl_dbg", [2, 1, 512], f32, isOutput=True)
        lb_dbg = nc.declare_dram_parameter("lb_dbg", [2, 64, 512], f32, isOutput=True)
        dbg_state = {"pt": [], "norm": []}

    with tile.TileContext(nc) as tc, ExitStack() as top:
        const = top.enter_context(tc.tile_pool(name="const", bufs=1))
        ones64 = const.tile([1, 64], f32, name="ones64")
        nc.vector.memset(ones64, 1.0)
        # tri2[:, h, :] = upper-triangular causal mask, replicated per head so
        # one DVE mul masks both heads of a pair
        tri2 = const.tile([128, 2, 128], bf16, name="tri2")
        nc.sync.dma_start(out=tri2[:, 0, :], in_=tri_d[:, :])
        nc.sync.dma_start(out=tri2[:, 1, :], in_=tri_d[:, :])
        # touch Exp early so the ~2.7us ACT table load overlaps phase A
        scr = const.tile([1, 8], bf16, name="scr")
        nc.scalar.activation(out=scr, in_=tri2[0:1, 0, 0:8], func=Exp, scale=1.0)

        big = top.enter_context(tc.tile_pool(name="big", bufs=1))
        # vaug[:, c, h, 0:64] = V[t=c*128..+128, e=h*64..+64]; col 64 = ones
        vaug = big.tile([128, NT, 8, 65], bf16, name="vaug")
        nc.vector.memset(vaug[:, :, :, 64:65], 1.0)

        xtp = top.enter_context(tc.tile_pool(name="xtp", bufs=1))
        xt = xtp.tile([128, NQ, ND, 512], bf16, name="xt")
        wvp = top.enter_context(tc.tile_pool(name="wvp", bufs=1))
        wv_sb = wvp.tile([128, ND, 512], bf16, name="wv_sb")
        qkp = top.enter_context(tc.tile_pool(name="qkp", bufs=1))
        qt = qkp.tile([128, NPAIR, T], bf16, name="qt")
        kt = qkp.tile([128, NPAIR, T], bf16, name="kt")
        otn_p = top.enter_context(tc.tile_pool(name="otn_p", bufs=1))
        otn = otn_p.tile([128, NPAIR, T], bf16, name="otn")
        pwo = top.enter_context(tc.tile_pool(name="pwo", bufs=1))
        wo_sb = pwo.tile([128, NPAIR, ND, 128], bf16, name="wo_sb")

        pw = top.enter_context(tc.tile_pool(name="pw", bufs=4))
        ptp = top.enter_context(tc.tile_pool(name="ptp", bufs=4))
        ocp = top.enter_context(tc.tile_pool(name="ocp", bufs=2))
        rcp = top.enter_context(tc.tile_pool(name="rcp", bufs=2))
        lbp = top.enter_context(tc.tile_pool(name="lbp", bufs=2))
        pyt = top.enter_context(tc.tile_pool(name="pyt", bufs=3))
        drp = top.enter_context(tc.tile_pool(name="drp", bufs=4, space="DRAM"))

        # PSUM budget: psS 2x2 banks + psO 2 + psM 2 = 8
        psS = top.enter_context(tc.tile_pool(name="psS", bufs=2, space="PSUM"))
        psO = top.enter_context(tc.tile_pool(name="psO", bufs=2, space="PSUM"))
        psM = top.enter_context(tc.tile_pool(name="psM", bufs=2, space="PSUM"))

        # ---- DMA in first-use order (startup is DMA-latency-bound) ---------
        wq_sbs = [None] * NPAIR
        wk_sbs = [None] * NPAIR

        def dma_w(wdram, p, kind):
            w_sb = pw.tile([128, ND, 128], bf16, tag="w", name=f"w_{kind}{p}")
            nc.sync.dma_start(out=w_sb, in_=wdram[p])
            return w_sb

        wq_sbs[0] = dma_w(wq_d, 0, "q")
        wk_sbs[0] = dma_w(wk_d, 0, "k")
        nc.sync.dma_start(out=xt[:, 0, :, :], in_=x_d[:, 0, :, :])
        nc.sync.dma_start(out=wv_sb, in_=wv_d[:, :, :])
        for t4 in range(1, NQ):
            nc.sync.dma_start(out=xt[:, t4, :, :], in_=x_d[:, t4, :, :])
        nc.sync.dma_start(out=wo_sb, in_=wo_d[:, :, :, :])

        # ---- building blocks ----------------------------------------------
        def qk_proj_mms(ps, w_sb, t4, dc_lo, dc_hi):
            for dc in range(dc_lo, dc_hi):
                nc.tensor.matmul(
                    ps, w_sb[:, dc, :], xt[:, t4, dc, :],
                    start=(dc == 0), stop=(dc == ND - 1),
                )

        def v_proj(tc_):
            """V[t, e] for one 128-token chunk, all 8 heads at once."""
            t4, sub = tc_ // 4, tc_ % 4
            ps = psM.tile([128, 512], f32, tag="mm", name="psv")
            for dc in range(ND):
                nc.tensor.matmul(
                    ps,
                    xt[:, t4, dc, sub * 128:(sub + 1) * 128],
                    wv_sb[:, dc, :],
                    start=(dc == 0), stop=(dc == ND - 1),
                )
            nc.vector.tensor_copy(out=vaug[:, tc_, :, 0:64], in_=ps)

        # ---- Phase A: Q/K pair 0 + V chunks 0-3 (t4=0 work first) ----------
        def qk0(w_sb, dest, t4):
            ps = psM.tile([128, 512], f32, tag="mm", name="psqk")
            qk_proj_mms(ps, w_sb, t4, 0, ND)
            nc.scalar.copy(out=dest[:, 0, t4 * 512:(t4 + 1) * 512], in_=ps)

        qk0(wq_sbs[0], qt, 0)
        qk0(wk_sbs[0], kt, 0)
        for tc_ in range(4):
            v_proj(tc_)

        # Deferred softmax-normalize multiplies: the 1/l partition-broadcast
        # rides a DRAM bounce; emitting the dependent DVE mul immediately
        # would head-of-line-block the in-order VectorE queue.  Each group
        # queues its two muls here; the next group flushes them (the DMA has
        # completed by then).
        pending_norm = []

        def flush_norm():
            while pending_norm:
                pending_norm.pop(0)()

        # ---- attention group for one head pair, one q-chunk ----------------
        def attn_group(p, j, filler):
            ncc = 4 * (j + 1)
            jw = j * 512
            flush_norm()
            po = [psO.tile([65, 512], f32, tag="O", name=f"po{h}")
                  for h in range(2)]
            pts = {}

            def off_of(c):
                sub = c - 4 * j
                return sub * 128 if 0 <= sub < 4 else 0

            def emit_s(c):
                # ps[:, h, :] spans two PSUM banks: the row-packed head
                # matmuls write different banks, one exp call reads both
                off = off_of(c)
                ps = psS.tile([128, 2, 512], f32, tag="S", name="ps")
                pt = ptp.tile([128, 2, 512], bf16, tag="pt", name="pt")
                for h in range(2):
                    e0 = h * 64
                    nc.tensor.matmul(
                        ps[:, h, off:],
                        kt[e0:e0 + 64, p, c * 128:(c + 1) * 128],
                        qt[e0:e0 + 64, p, jw + off:jw + 512],
                        start=True, stop=True,
                    )
                nc.scalar.activation(out=pt[:, :, off:], in_=ps[:, :, off:],
                                     func=Exp, scale=0.125)
                sub = c - 4 * j
                if 0 <= sub < 4:
                    nc.vector.tensor_mul(
                        pt[:, :, sub * 128:(sub + 1) * 128],
                        pt[:, :, sub * 128:(sub + 1) * 128],
                        tri2,
                    )
                if dbg and p == 0 and j == 0:
                    nc.sync.dma_start(out=pt_dbg[c], in_=pt)
                pts[c] = pt

            def emit_v(c):
                pt = pts.pop(c)
                off = off_of(c)
                for h in range(2):
                    nc.tensor.matmul(
                        po[h][:, off:],
                        vaug[:, c, 2 * p + h, :],
                        pt[:, h, off:],
                        start=(c == 0), stop=(c == ncc - 1),
                    )

            emit_s(0)
            if ncc > 1:
                emit_s(1)
            for c in range(ncc):
                if c + 2 < ncc:
                    emit_s(c + 2)
                if c % 2 == 0:
                    filler()
                emit_v(c)

            # normalize: otn[e, q] = O_T[e, q] / l[q] on DVE;
            # the final mul is deferred until the broadcast DMA has landed
            for h in range(2):
                oc = ocp.tile([64, 512], bf16, tag="oc", name="oc")
                nc.vector.tensor_copy(out=oc, in_=po[h][0:64, :])
                rlc = rcp.tile([1, 512], f32, tag="rlc", name="rlc")
                nc.vector.tensor_copy(out=rlc, in_=po[h][64:65, :])
                rl = rcp.tile([1, 512], f32, tag="rl", name="rl")
                nc.vector.reciprocal_approx_fast(rl, rlc)
                lb = lbp.tile([64, 512], f32, tag="lb", name="lb")
                nc.gpsimd.partition_broadcast(lb, rl)
                e0 = h * 64

                def norm_mul(oc=oc, lb=lb, e0=e0, p=p, jw=jw):
                    nc.vector.tensor_mul(
                        otn[e0:e0 + 64, p, jw:jw + 512], oc, lb
                    )
                pending_norm.append(norm_mul)
                if dbg and p == 0 and j == 0:
                    nc.sync.dma_start(out=oc_dbg[h], in_=oc)
                    nc.sync.dma_start(out=rl_dbg[h], in_=rl)
                    nc.sync.dma_start(out=lb_dbg[h], in_=lb)

        # ---- filler units ---------------------------------------------------
        def mk_qk_unit(w_sb, dest, p, t4, dc_lo, dc_hi, state):
            def emit():
                if dc_lo == 0:
                    state["ps"] = psM.tile([128, 512], f32, tag="mm", name="psf")
                qk_proj_mms(state["ps"], w_sb, t4, dc_lo, dc_hi)
                if dc_hi == ND:
                    nc.vector.tensor_copy(
                        out=dest[:, p, t4 * 512:(t4 + 1) * 512],
                        in_=state["ps"])
            return emit

        def mk_out_unit(dc, qc, tail=False):
            def emit():
                py = psM.tile([128, 512], f32, tag="mm", name="pyo")
                for pp in range(NPAIR):
                    nc.tensor.matmul(
                        py,
                        wo_sb[:, pp, dc, :],
                        otn[:, pp, qc * 512:(qc + 1) * 512],
                        start=(pp == 0), stop=(pp == NPAIR - 1),
                    )
                yt_sb = pyt.tile([128, 512], bf16, tag="yt", name="yt_f")
                if tail:
                    nc.scalar.copy(out=yt_sb, in_=py)
                else:
                    nc.vector.tensor_copy(out=yt_sb, in_=py)
                nc.sync.dma_start(
                    out=yt_d[dc * 128:(dc + 1) * 128,
                             qc * 512:(qc + 1) * 512],
                    in_=yt_sb,
                )
            return emit

        # ---- Phase B: four pair phases -------------------------------------
        for p in range(NPAIR):
            fill = []
            if p == 0:
                # pair-0 j>0 needs qt/kt t4=j first, then remaining V chunks
                # (rate-2 filler covers readiness), then QK pair 1
                for t4 in range(1, NQ):
                    fill.append(lambda t4=t4: qk0(wq_sbs[0], qt, t4))
                    fill.append(lambda t4=t4: qk0(wk_sbs[0], kt, t4))
                for tc_ in range(4, NT):
                    fill.append(lambda tc_=tc_: v_proj(tc_))
            if p < NPAIR - 1:
                wq_sbs[p + 1] = dma_w(wq_d, p + 1, "q")
                wk_sbs[p + 1] = dma_w(wk_d, p + 1, "k")
                for w_sb, dest in ((wq_sbs[p + 1], qt), (wk_sbs[p + 1], kt)):
                    for t4 in range(NQ):
                        state = {}
                        for dc_lo in (0, 4):
                            fill.append(mk_qk_unit(w_sb, dest, p + 1, t4,
                                                   dc_lo, dc_lo + 4, state))

            def filler(fill=fill):
                n = 2 if fill and len(fill) > 8 else 1
                for _ in range(n):
                    if fill:
                        fill.pop(0)()

            if p < NPAIR - 1:
                for j in range(NQ):
                    attn_group(p, j, filler)
            else:
                # pair 3: j descending, so the tail waits only on the
                # shortest group's normalize; out-units follow completions
                for j in (3, 2, 1, 0):
                    if j < 3:
                        qc = j + 1
                        for dc in range(ND):
                            fill.append(mk_out_unit(dc, qc))
                    attn_group(p, j, filler)
            while fill:
                fill.pop(0)()

        # tail: q-chunk 0 of the output projection (copies on idle ScalarE)
        flush_norm()
        for dc in range(ND):
            mk_out_unit(dc, 0, tail=True)()

        if dbg:
            nc.sync.dma_start(out=qt_dbg[:, :, :], in_=qt)
            nc.sync.dma_start(out=kt_dbg[:, :, :], in_=kt)
            nc.sync.dma_start(out=va_dbg[:, :, :, :], in_=vaug)
            nc.sync.dma_start(out=ot_dbg[:, :, :], in_=otn)

    nc.compile()
    return nc


def _pack_inputs(x, Wq, Wk, Wv, Wo):
    """Per-core input maps. Core c: batch c//2, head group c%2."""
    import ml_dtypes

    tri = np.triu(np.ones((128, 128), np.float32)).astype(ml_dtypes.bfloat16)

    def pack_w(W, g):
        # [NPAIR, 128(d_local), ND, 128(e2)]
        out = np.empty((NPAIR, 128, ND, 128), np.float32)
        for p in range(NPAIR):
            h1 = 8 * g + 2 * p
            r = W[[h1, h1 + 1]].transpose(1, 0, 2).reshape(D, 128)  # [d, e2]
            out[p] = r.reshape(ND, 128, 128).transpose(1, 0, 2)
        return np.ascontiguousarray(out).astype(ml_dtypes.bfloat16)

    def pack_wv(W, g):
        # [128(d within chunk), ND, 512(e = h*64+hs over 8 heads)]
        r = W[8 * g:8 * g + 8].transpose(1, 0, 2).reshape(D, 512)  # [d, e]
        out = r.reshape(ND, 128, 512).transpose(1, 0, 2)
        return np.ascontiguousarray(out).astype(ml_dtypes.bfloat16)

    def pack_wo(Wo, g):
        # [128(e2), NPAIR, ND, 128(d)]
        out = np.empty((128, NPAIR, ND, 128), np.float32)
        for p in range(NPAIR):
            r0 = (8 * g + 2 * p) * 64
            out[:, p] = Wo[r0:r0 + 128].reshape(128, ND, 128)
        return np.ascontiguousarray(out).astype(ml_dtypes.bfloat16)

    packs = {}
    for g in range(2):
        packs[g] = dict(
            wq=pack_w(Wq, g), wk=pack_w(Wk, g), wv=pack_wv(Wv, g),
            wo=pack_wo(Wo, g),
        )
    in_maps = []
    for c in range(NCORES):
        b, g = c // 2, c % 2
        m = dict(packs[g])
        xt = x[b].reshape(NQ, 512, ND, 128).transpose(3, 0, 2, 1)
        m["x"] = np.ascontiguousarray(xt).astype(ml_dtypes.bfloat16)
        m["tri"] = tri
        in_maps.append(m)
    return in_maps


def kernel(x, Wq, Wk, Wv, Wo, bo):
    from concourse.bass_utils import run_bass_kernel_spmd

    x = np.asarray(x, np.float32)
    Wq, Wk, Wv = (np.asarray(a, np.float32) for a in (Wq, Wk, Wv))
    Wo = np.asarray(Wo, np.float32)
    bo = np.asarray(bo, np.float32)

    if "nc" not in _CACHE:
        _CACHE["nc"] = _build_program()
    nc = _CACHE["nc"]

    in_maps = _pack_inputs(x, Wq, Wk, Wv, Wo)
    res = run_bass_kernel_spmd(nc, in_maps, list(range(NCORES)))
    _CACHE["last_result"] = res

    out = np.empty((B, T, D), np.float32)
    for b in range(B):
        yt = (res.results[2 * b]["yt"].astype(np.float32)
              + res.results[2 * b + 1]["yt"].astype(np.float32))
        out[b] = yt.T + bo
    return out


# revision 24
# speedup vs baseline: 1.2441x; 1.0086x over previous
"""Multi-head causal attention (B=4, T=2048, D=1024, H=16, HS=64) on 8 TRN2
NeuronCores.

Sharding: batch (4-way) x head-group (2-way).  Core c handles batch c//2 and
heads 8*(c%2) .. 8*(c%2)+7.  Each core computes its 8 heads' attention and the
partial output projection Y_T = sum_h Wo_h^T @ O_T_h; the host sums the two
head-group partials per batch, transposes, and adds the output bias.

v2 structure (vs the v1 baseline):
  - V is produced directly in [t, e] layout (lhsT = x^T chunks, rhs = all 8
    heads' Wv columns), which removes all 128 PE transposes.
  - Scores S^T [k, q] contract over e=64, so the two heads of a pair are
    row-packed: head 0 on PE rows 0-63, head 1 on rows 64-127 (tile_position
    auto-derived from base partitions).  Adjacent emission makes the two
    matmuls run concurrently -> ~2x on the S stream.
  - S lands in PSUM as bf16: one 2KB bank holds a [128, 2, 512] slot (two
    k-chunks), so exp runs as [128, ~1024] activations and the whole
    attention pipeline fits in 8 banks: 4 x S (2 heads x 2 slots in flight)
    + 2 x O accumulators + 2 x general matmul banks.
  - The output projection runs as filler inside pair 3's attention: for each
    (dc, qc) all four pair contributions accumulate in one PSUM bank, then a
    single copy + DMA out.  Q/K projections for pair p+1 and the remaining
    V-projection chunks fill pairs 0-2.
  - ScalarE does (almost) only exp; psum evacuations and the softmax
    normalization (1/l broadcast via a DRAM bounce) run on VectorE in bf16.
"""

import numpy as np

B, T, D = 4, 2048, 1024
H, HS = 16, 64
NCORES = 8
NPAIR = 4   # head pairs per core
ND = 8      # 128-wide d chunks
NT = 16     # 128-wide t chunks
NQ = 4      # 512-wide q chunks
NK = 16     # 128-wide k chunks

_CACHE = {}


def _build_program(dbg=False):
    import concourse.bass as bass
    import concourse.tile as tile
    from concourse import bacc, mybir
    from contextlib import ExitStack

    f32 = mybir.dt.float32
    bf16 = mybir.dt.bfloat16
    f8 = mybir.dt.float8e4
    Exp = mybir.ActivationFunctionType.Exp
    DR = mybir.MatmulPerfMode.DoubleRow

    nc = bacc.Bacc("TRN2", target_bir_lowering=False, debug=False)

    x_d = nc.declare_dram_parameter("x", [128, NQ, ND, 512], bf16, isOutput=False)
    wq_d = nc.declare_dram_parameter("wq", [NPAIR, 128, ND, 128], bf16, isOutput=False)
    wk_d = nc.declare_dram_parameter("wk", [NPAIR, 128, ND, 128], bf16, isOutput=False)
    wv_d = nc.declare_dram_parameter("wv", [128, ND, 512], bf16, isOutput=False)
    wo_d = nc.declare_dram_parameter("wo", [128, NPAIR, ND, 128], bf16, isOutput=False)
    tri_d = nc.declare_dram_parameter("tri", [128, 128], bf16, isOutput=False)
    yt_d = nc.declare_dram_parameter("yt", [D, T], bf16, isOutput=True)
    if dbg:
        qt_dbg = nc.declare_dram_parameter("qt_dbg", [128, NPAIR, T], bf16, isOutput=True)
        kt_dbg = nc.declare_dram_parameter("kt_dbg", [128, NPAIR, T], bf16, isOutput=True)
        va_dbg = nc.declare_dram_parameter("va_dbg", [128, NT, 8, 65], bf16, isOutput=True)
        ot_dbg = nc.declare_dram_parameter("ot_dbg", [128, NPAIR, T], bf16, isOutput=True)
        pt_dbg = nc.declare_dram_parameter("pt_dbg", [4, 128, 2, 512], bf16, isOutput=True)
        oc_dbg = nc.declare_dram_parameter("oc_dbg", [2, 64, 512], f32, isOutput=True)
        rl_dbg = nc.declare_dram_parameter("rl_dbg", [2, 1, 512], f32, isOutput=True)
        lb_dbg = nc.declare_dram_parameter("lb_dbg", [2, 64, 512], f32, isOutput=True)
        dbg_state = {"pt": [], "norm": []}

    with tile.TileContext(nc) as tc, ExitStack() as top:
        const = top.enter_context(tc.tile_pool(name="const", bufs=1))
        ones64 = const.tile([1, 64], f32, name="ones64")
        nc.vector.memset(ones64, 1.0)
        # tri2[:, h, :] = upper-triangular causal mask, replicated per head so
        # one DVE mul masks both heads of a pair
        tri2 = const.tile([128, 2, 128], bf16, name="tri2")
        nc.sync.dma_start(out=tri2[:, 0, :], in_=tri_d[:, :])
        nc.sync.dma_start(out=tri2[:, 1, :], in_=tri_d[:, :])
        # touch Exp early so the ~2.7us ACT table load overlaps phase A
        scr = const.tile([1, 8], bf16, name="scr")
        nc.scalar.activation(out=scr, in_=tri2[0:1, 0, 0:8], func=Exp, scale=1.0)

        big = top.enter_context(tc.tile_pool(name="big", bufs=1))
        # vaug[:, c, h, 0:64] = V[t=c*128..+128, e=h*64..+64]; col 64 = ones
        vaug = big.tile([128, NT, 8, 65], bf16, name="vaug")
        nc.vector.memset(vaug[:, :, :, 64:65], 1.0)

        xtp = top.enter_context(tc.tile_pool(name="xtp", bufs=1))
        xt = xtp.tile([128, NQ, ND, 512], bf16, name="xt")
        wvp = top.enter_context(tc.tile_pool(name="wvp", bufs=1))
        wv_sb = wvp.tile([128, ND, 512], bf16, name="wv_sb")
        qkp = top.enter_context(tc.tile_pool(name="qkp", bufs=1))
        qt = qkp.tile([128, NPAIR, T], bf16, name="qt")
        kt = qkp.tile([128, NPAIR, T], bf16, name="kt")
        otn_p = top.enter_context(tc.tile_pool(name="otn_p", bufs=1))
        otn = otn_p.tile([128, NPAIR, T], bf16, name="otn")
        pwo = top.enter_context(tc.tile_pool(name="pwo", bufs=1))
        wo_sb = pwo.tile([128, NPAIR, ND, 128], bf16, name="wo_sb")

        pw = top.enter_context(tc.tile_pool(name="pw", bufs=4))
        ptp = top.enter_context(tc.tile_pool(name="ptp", bufs=4))
        ocp = top.enter_context(tc.tile_pool(name="ocp", bufs=4))
        rcp = top.enter_context(tc.tile_pool(name="rcp", bufs=4))
        lbp = top.enter_context(tc.tile_pool(name="lbp", bufs=4))
        pyt = top.enter_context(tc.tile_pool(name="pyt", bufs=3))
        drp = top.enter_context(tc.tile_pool(name="drp", bufs=4, space="DRAM"))

        # PSUM budget: psS 2x2 banks + psO 2 + psM 2 = 8
        psS = top.enter_context(tc.tile_pool(name="psS", bufs=2, space="PSUM"))
        psO = top.enter_context(tc.tile_pool(name="psO", bufs=2, space="PSUM"))
        psM = top.enter_context(tc.tile_pool(name="psM", bufs=2, space="PSUM"))

        # ---- DMA in first-use order (startup is DMA-latency-bound) ---------
        wq_sbs = [None] * NPAIR
        wk_sbs = [None] * NPAIR

        def dma_w(wdram, p, kind):
            w_sb = pw.tile([128, ND, 128], bf16, tag="w", name=f"w_{kind}{p}")
            nc.sync.dma_start(out=w_sb, in_=wdram[p])
            return w_sb

        wq_sbs[0] = dma_w(wq_d, 0, "q")
        wk_sbs[0] = dma_w(wk_d, 0, "k")
        nc.sync.dma_start(out=xt[:, 0, :, :], in_=x_d[:, 0, :, :])
        nc.sync.dma_start(out=wv_sb, in_=wv_d[:, :, :])
        for t4 in range(1, NQ):
            nc.sync.dma_start(out=xt[:, t4, :, :], in_=x_d[:, t4, :, :])
        nc.sync.dma_start(out=wo_sb, in_=wo_d[:, :, :, :])

        # ---- building blocks ----------------------------------------------
        def qk_proj_mms(ps, w_sb, t4, dc_lo, dc_hi):
            for dc in range(dc_lo, dc_hi):
                nc.tensor.matmul(
                    ps, w_sb[:, dc, :], xt[:, t4, dc, :],
                    start=(dc == 0), stop=(dc == ND - 1),
                )

        def v_proj(tc_):
            """V[t, e] for one 128-token chunk, all 8 heads at once."""
            t4, sub = tc_ // 4, tc_ % 4
            ps = psM.tile([128, 512], f32, tag="mm", name="psv")
            for dc in range(ND):
                nc.tensor.matmul(
                    ps,
                    xt[:, t4, dc, sub * 128:(sub + 1) * 128],
                    wv_sb[:, dc, :],
                    start=(dc == 0), stop=(dc == ND - 1),
                )
            nc.vector.tensor_copy(out=vaug[:, tc_, :, 0:64], in_=ps)

        # ---- Phase A: Q/K pair 0 + V chunks 0-3 (t4=0 work first) ----------
        def qk0(w_sb, dest, t4):
            ps = psM.tile([128, 512], f32, tag="mm", name="psqk")
            qk_proj_mms(ps, w_sb, t4, 0, ND)
            nc.scalar.copy(out=dest[:, 0, t4 * 512:(t4 + 1) * 512], in_=ps)

        qk0(wq_sbs[0], qt, 0)
        qk0(wk_sbs[0], kt, 0)
        for tc_ in range(4):
            v_proj(tc_)

        # Deferred softmax-normalize multiplies: the 1/l partition-broadcast
        # has ~1us latency on GpSimd; emitting the dependent DVE mul
        # immediately would head-of-line-block the in-order VectorE queue.
        # Muls queue here and flush a full group later; pair-3 out-proj units
        # enter the filler queue only after their muls are emitted.
        pending_norm = []
        unit_backlog = []
        cur_fill = [None]

        def flush_norm():
            while pending_norm:
                pending_norm.pop(0)()
            if cur_fill[0] is not None and unit_backlog:
                cur_fill[0].extend(unit_backlog)
                del unit_backlog[:]

        def normalize(p, j, po):
            jw = j * 512
            for h in range(2):
                oc = ocp.tile([64, 512], bf16, tag="oc", name="oc")
                nc.vector.tensor_copy(out=oc, in_=po[h][0:64, :])
                rlc = rcp.tile([1, 512], f32, tag="rlc", name="rlc")
                nc.vector.tensor_copy(out=rlc, in_=po[h][64:65, :])
                rl = rcp.tile([1, 512], f32, tag="rl", name="rl")
                nc.vector.reciprocal_approx_fast(rl, rlc)
                lb = lbp.tile([64, 512], f32, tag="lb", name="lb")
                nc.gpsimd.partition_broadcast(lb, rl)
                e0 = h * 64

                def norm_mul(oc=oc, lb=lb, e0=e0, p=p, jw=jw):
                    nc.vector.tensor_mul(
                        otn[e0:e0 + 64, p, jw:jw + 512], oc, lb
                    )
                pending_norm.append(norm_mul)

        # ---- attention for one head pair: all (j, c) chunks as one stream --
        def attn_pair(p, filler, on_group_done=None):
            chunks = []
            for j in range(NQ):
                for c in range(4 * (j + 1)):
                    chunks.append((j, c))
            pts = {}
            po_of = {}

            def off_of(j, c):
                sub = c - 4 * j
                return sub * 128 if 0 <= sub < 4 else 0

            def emit_s(j, c):
                # flush points: deferred muls are a full group old here
                if c == 0 or (j == NQ - 1 and c == 8):
                    flush_norm()
                # ps[:, h, :] spans two PSUM banks: the row-packed head
                # matmuls write different banks, one exp call reads both
                off = off_of(j, c)
                jw = j * 512
                ps = psS.tile([128, 2, 512], f32, tag="S", name="ps")
                pt = ptp.tile([128, 2, 512], bf16, tag="pt", name="pt")
                for h in range(2):
                    e0 = h * 64
                    nc.tensor.matmul(
                        ps[:, h, off:],
                        kt[e0:e0 + 64, p, c * 128:(c + 1) * 128],
                        qt[e0:e0 + 64, p, jw + off:jw + 512],
                        start=True, stop=True,
                    )
                nc.scalar.activation(out=pt[:, :, off:], in_=ps[:, :, off:],
                                     func=Exp, scale=0.125)
                sub = c - 4 * j
                if 0 <= sub < 4:
                    nc.vector.tensor_mul(
                        pt[:, :, sub * 128:(sub + 1) * 128],
                        pt[:, :, sub * 128:(sub + 1) * 128],
                        tri2,
                    )
                pts[(j, c)] = pt

            def emit_v(j, c):
                ncc = 4 * (j + 1)
                if c == 0:
                    po_of[j] = [psO.tile([65, 512], f32, tag="O", name=f"po{h}")
                                for h in range(2)]
                po = po_of[j]
                pt = pts.pop((j, c))
                off = off_of(j, c)
                for h in range(2):
                    nc.tensor.matmul(
                        po[h][:, off:],
                        vaug[:, c, 2 * p + h, :],
                        pt[:, h, off:],
                        start=(c == 0), stop=(c == ncc - 1),
                    )
                if c == ncc - 1:
                    normalize(p, j, po_of.pop(j))
                    if on_group_done is not None:
                        on_group_done(j)

            emit_s(*chunks[0])
            emit_s(*chunks[1])
            for idx, (j, c) in enumerate(chunks):
                if idx + 2 < len(chunks):
                    emit_s(*chunks[idx + 2])
                if idx % 2 == 0:
                    filler()
                emit_v(j, c)

        # ---- filler units ---------------------------------------------------
        def mk_qk_unit(w_sb, dest, p, t4, dc_lo, dc_hi, state):
            def emit():
                if dc_lo == 0:
                    state["ps"] = psM.tile([128, 512], f32, tag="mm", name="psf")
                qk_proj_mms(state["ps"], w_sb, t4, dc_lo, dc_hi)
                if dc_hi == ND:
                    nc.vector.tensor_copy(
                        out=dest[:, p, t4 * 512:(t4 + 1) * 512],
                        in_=state["ps"])
            return emit

        def mk_out_unit(dc, qc, tail=False):
            def emit():
                py = psM.tile([128, 512], f32, tag="mm", name="pyo")
                for pp in range(NPAIR):
                    nc.tensor.matmul(
                        py,
                        wo_sb[:, pp, dc, :],
                        otn[:, pp, qc * 512:(qc + 1) * 512],
                        start=(pp == 0), stop=(pp == NPAIR - 1),
                    )
                yt_sb = pyt.tile([128, 512], bf16, tag="yt", name="yt_f")
                if tail:
                    nc.scalar.copy(out=yt_sb, in_=py)
                else:
                    nc.vector.tensor_copy(out=yt_sb, in_=py)
                nc.sync.dma_start(
                    out=yt_d[dc * 128:(dc + 1) * 128,
                             qc * 512:(qc + 1) * 512],
                    in_=yt_sb,
                )
            return emit

        # ---- Phase B: four pair phases -------------------------------------
        for p in range(NPAIR):
            fill = []
            cur_fill[0] = fill
            if p == 0:
                # pair-0 j>0 needs qt/kt t4=j first, then remaining V chunks
                # (rate-2 filler covers readiness), then QK pair 1
                for t4 in range(1, NQ):
                    fill.append(lambda t4=t4: qk0(wq_sbs[0], qt, t4))
                    fill.append(lambda t4=t4: qk0(wk_sbs[0], kt, t4))
                for tc_ in range(4, NT):
                    fill.append(lambda tc_=tc_: v_proj(tc_))
            if p < NPAIR - 1:
                wq_sbs[p + 1] = dma_w(wq_d, p + 1, "q")
                wk_sbs[p + 1] = dma_w(wk_d, p + 1, "k")
                for w_sb, dest in ((wq_sbs[p + 1], qt), (wk_sbs[p + 1], kt)):
                    for t4 in range(NQ):
                        state = {}
                        for dc_lo in (0, 4):
                            fill.append(mk_qk_unit(w_sb, dest, p + 1, t4,
                                                   dc_lo, dc_lo + 4, state))

            def filler(fill=fill):
                n = 2 if fill and len(fill) > 8 else 1
                for _ in range(n):
                    if fill:
                        fill.pop(0)()

            if p < NPAIR - 1:
                attn_pair(p, filler)
            else:
                def on_done(j):
                    for dc in range(ND):
                        unit_backlog.append(mk_out_unit(dc, j, tail=(j == 3)))
                attn_pair(p, filler, on_group_done=on_done)
            while fill:
                fill.pop(0)()

        # tail: drain remaining out-proj units (muls flushed first)
        flush_norm()
        while fill:
            fill.pop(0)()

        if dbg:
            nc.sync.dma_start(out=qt_dbg[:, :, :], in_=qt)
            nc.sync.dma_start(out=kt_dbg[:, :, :], in_=kt)
            nc.sync.dma_start(out=va_dbg[:, :, :, :], in_=vaug)
            nc.sync.dma_start(out=ot_dbg[:, :, :], in_=otn)

    nc.compile()
    return nc


def _pack_inputs(x, Wq, Wk, Wv, Wo):
    """Per-core input maps. Core c: batch c//2, head group c%2."""
    import ml_dtypes

    tri = np.triu(np.ones((128, 128), np.float32)).astype(ml_dtypes.bfloat16)

    def pack_w(W, g):
        # [NPAIR, 128(d_local), ND, 128(e2)]
        out = np.empty((NPAIR, 128, ND, 128), np.float32)
        for p in range(NPAIR):
            h1 = 8 * g + 2 * p
            r = W[[h1, h1 + 1]].transpose(1, 0, 2).reshape(D, 128)  # [d, e2]
            out[p] = r.reshape(ND, 128, 128).transpose(1, 0, 2)
        return np.ascontiguousarray(out).astype(ml_dtypes.bfloat16)

    def pack_wv(W, g):
        # [128(d within chunk), ND, 512(e = h*64+hs over 8 heads)]
        r = W[8 * g:8 * g + 8].transpose(1, 0, 2).reshape(D, 512)  # [d, e]
        out = r.reshape(ND, 128, 512).transpose(1, 0, 2)
        return np.ascontiguousarray(out).astype(ml_dtypes.bfloat16)

    def pack_wo(Wo, g):
        # [128(e2), NPAIR, ND, 128(d)]
        out = np.empty((128, NPAIR, ND, 128), np.float32)
        for p in range(NPAIR):
            r0 = (8 * g + 2 * p) * 64
            out[:, p] = Wo[r0:r0 + 128].reshape(128, ND, 128)
        return np.ascontiguousarray(out).astype(ml_dtypes.bfloat16)

    packs = {}
    for g in range(2):
        packs[g] = dict(
            wq=pack_w(Wq, g), wk=pack_w(Wk, g), wv=pack_wv(Wv, g),
            wo=pack_wo(Wo, g),
        )
    in_maps = []
    for c in range(NCORES):
        b, g = c // 2, c % 2
        m = dict(packs[g])
        xt = x[b].reshape(NQ, 512, ND, 128).transpose(3, 0, 2, 1)
        m["x"] = np.ascontiguousarray(xt).astype(ml_dtypes.bfloat16)
        m["tri"] = tri
        in_maps.append(m)
    return in_maps


def kernel(x, Wq, Wk, Wv, Wo, bo):
    from concourse.bass_utils import run_bass_kernel_spmd

    x = np.asarray(x, np.float32)
    Wq, Wk, Wv = (np.asarray(a, np.float32) for a in (Wq, Wk, Wv))
    Wo = np.asarray(Wo, np.float32)
    bo = np.asarray(bo, np.float32)

    if "nc" not in _CACHE:
        _CACHE["nc"] = _build_program()
    nc = _CACHE["nc"]

    in_maps = _pack_inputs(x, Wq, Wk, Wv, Wo)
    res = run_bass_kernel_spmd(nc, in_maps, list(range(NCORES)))
    _CACHE["last_result"] = res

    out = np.empty((B, T, D), np.float32)
    for b in range(B):
        yt = (res.results[2 * b]["yt"].astype(np.float32)
              + res.results[2 * b + 1]["yt"].astype(np.float32))
        out[b] = yt.T + bo
    return out
